# revision 18
# baseline (speedup 1.0000x reference)
"""Trainium2 Bass kernel for an ExponentialRNN (modrelu recurrence).

Computation (per example b):
    xT = x @ T                                   # [B, S, U] pre-projection
    h_{t+1} = modrelu(xT[:, t] + h_t @ B, bias)  # 512 sequential steps
    out[t] = h_{t+1}                             # [S, B, U]

Fast path (always taken for the reference inputs): B = expm(A) in the
reference is EXACTLY block-diagonal with 2x2 rotation blocks acting on
unit pairs (2p, 2p+1), so h @ B is an elementwise pair rotation:

    z[2p]   = h[2p]*B[2p,2p]   + h[2p+1]*B[2p+1,2p]
    z[2p+1] = h[2p]*B[2p,2p+1] + h[2p+1]*B[2p+1,2p+1]

Sharding: batch 4-way x pairs 2-way across the 8 cores (16 examples x
128 pairs per core), chosen so each recurrence step is exactly 4 DVE ops
of [128, 16] (the structural floor: each output parity needs 3 tensor
inputs -- two state halves + x-tilde -- on a 2-src ISA):

    t_e = we + (A10/A00)*wo             (native scalar_tensor_tensor)
    t_o = wo + (A01/A11)*we             (native scalar_tensor_tensor)
    we' = modrelu(A00*t_e + xtt_e, be)  (MRS custom op, signed bias)
    wo' = modrelu(A11*t_o + xtt_o, bo)  (MRS custom op)

using modrelu(z, b) = select(z>=0, max(z+b, 0), min(z-b, 0)), exact for
either sign of b.  Native STT ops measure ~190 ns vs ~250 ns for a
2-scalar custom; customs cost ~204 ns flat regardless of width/uop count.
State lives in the fp32 output archive ybuf[g][:, 16t:16t+16] (no copies;
same-engine program order needs no semaphores; ~860 ns/step, DVE ~95%
busy).  Pair layout keeps both pair members on the same partition (DVE
lanes cannot cross partitions).  x is streamed in per 512-col block; the
pre-projection runs on PE (fp32 matmuls) interleaved two blocks ahead of
the recurrence, with psum->SBUF copies on the Act engine so the DVE
stream stays dedicated to the recurrence.  The archive is drained to HBM
at the end; the host reassembles/unpermutes.

Measured on trn2 (8 cores): ~446 us (+-1.5 us run-to-run) vs 3106 us
for the dense-matmul baseline (kept below as the fallback when B is not
block-diagonal).  Remaining step-boundary penalty (~125 ns/step): custom
DVE ops have ~125 ns write-back latency (native writes are immediately
readable), and the first STT of each step necessarily reads a state
half written by a custom MRS one instruction earlier -- every
linearization of the step's dependency cycle has exactly one such edge.
KERNEL_IMPL selects dve2 (default) / dve (2-scalar-custom rotation,
also used when a block diagonal entry is ~0) / pe / dense.
"""


import os
import sys

import numpy as np

for _p in ("/opt/trn_rl_repo", "/root/.axon_site/_ro/trn_rl_repo"):
    if os.path.isdir(_p) and _p not in sys.path:
        sys.path.insert(0, _p)

import concourse.bass as bass
import concourse.bacc as bacc
import concourse.mybir as mybir
import concourse.bass_utils as bass_utils
import concourse.dve_ops as dve_ops
from concourse.dve_spec import Spec, Src0, Src1, C0, C1, Zero, maxx, minn, select, lower
from concourse.dve_uop import DveOpSpec
from concourse.tile import TileContext

BATCH, SEQ, DIN, UNITS = 64, 512, 256, 512
NCORES = 8
BS = BATCH // NCORES          # per-core batch = 8
NK = UNITS // 128             # 4 unit chunks
ND = DIN // 128               # 2 din chunks
F32 = mybir.dt.float32
F32R = mybir.dt.float32r
TB = SEQ * BS                 # flattened (t, b) = 4096
NPAIR = UNITS // 2            # 256 rotation pairs
NG = 4                        # unit groups of 128 (even/odd x pair-chunk)


LAST_RESULTS = None


def _register_dve_op(name, body, ref):
    """Register a custom DVE op (idempotent)."""
    for op in dve_ops.OPS:
        if op.name == name:
            return op
    spec = Spec(body=body, reference=ref)
    shas = {}
    for ver in ("v3", "v4"):
        try:
            uops = lower(spec, ver=ver)
        except Exception:
            continue
        shas[ver] = DveOpSpec(name=name, uops=uops, rd1_en=True).sha(ver)
    op = dve_ops.DveOp(name, spec, subdim=False, uops_sha=shas)
    dve_ops.OPS.append(op)
    row = max(dve_ops._SUB_OPCODE_FOR_NAME.values()) + 1
    assert row < 0x20, "custom DVE opcode rows exhausted"
    dve_ops._SUB_OPCODE_FOR_NAME[name] = row
    dve_ops.CUSTOM_DVE_SPECS[name] = spec
    return op


def _register_modrelu():
    """out = z + clamp(z*C0, C1, -C1)  with z = Src0 + Src1
    equals sign(z) * relu(|z| + bias) for
      C0 = bias >= 0 ? BIG : -1.0 ,  C1 = |bias|   (per-partition scalars)."""
    z = Src0 + Src1
    return _register_dve_op(
        "MODRELU_STEP_ANT",
        z + maxx(minn(z * C0, C1), Zero - C1),
        lambda in0, in1, s0, s1, imm2: (in0 + in1)
        + np.maximum(np.minimum((in0 + in1) * s0, s1), -s1),
    )


def _register_rot():
    """out = Src0*C0 + Src1*C1 — one half of a 2x2 pair rotation."""
    return _register_dve_op(
        "PAIR_ROT_ANT",
        Src0 * C0 + Src1 * C1,
        lambda in0, in1, s0, s1, imm2: in0 * s0 + in1 * s1,
    )


def _register_mrs():
    """out = modrelu(Src0*C0 + Src1, C1) with SIGNED bias C1 and per-unit
    prescale C0, via the branch identity
      modrelu(z, b) = select(z>=0, max(z+b, 0), min(z-b, 0))
    (exact for all signs of b; differs from sign(z)*relu(|z|+b) only on the
    measure-zero event z == 0 exactly)."""
    z = Src0 * C0 + Src1
    return _register_dve_op(
        "MODRELU_PRESCALE_SEL_ANT",
        select(z >= Zero, maxx(z + C1, Zero), minn(z - C1, Zero)),
        lambda in0, in1, s0, s1, imm2: np.where(
            (in0 * s0 + in1) >= 0,
            np.maximum(in0 * s0 + in1 + s1, 0),
            np.minimum(in0 * s0 + in1 - s1, 0)),
    )


MODRELU = _register_modrelu()
ROT = _register_rot()
MRS = _register_mrs()

_NC_CACHE = {}


# ---------------------------------------------------------------------------
# Fast path: block-diagonal rotation recurrence (pure DVE)
#
# Sharding: batch 4-way x pairs 2-way across the 8 cores.  Core c handles
# batch shard cb = c // 2 (16 examples) and pair shard cp = c % 2 (128
# pairs = 1 full partition chunk), so each recurrence step is exactly 4
# DVE ops of [128, 16]: rot-e, rot-o, modrelu-e, modrelu-o.
# ---------------------------------------------------------------------------

NBS = 2                       # pair shards
BSR = BATCH // (NCORES // NBS)  # examples per core = 16
TBR = SEQ * BSR               # per-core flattened (t, b) = 8192
NJR = TBR // 512              # 16 col-blocks of 512 (= 32 steps each)


def _build_nc_rot_pe():
    """PE-assisted recurrence: per step, 2 matmuls rotate the state via the
    block-diagonal B chunk stationaries (f32r), and 2 two-src MODRELU ops on
    DVE read the psum rotation + SBUF x-tilde and write the f32r state
    archive.  Two independent chains (one per 128-unit chunk) pipeline
    across PE and DVE; measured chain latency ~550 ns/step."""
    nc = bacc.Bacc()

    xtr_d = nc.dram_tensor("xtr", [DIN, TBR], F32, kind="ExternalInput")
    t_d = nc.dram_tensor("tmat", [DIN, 256], F32, kind="ExternalInput")
    b_d = nc.dram_tensor("bmat", [256, 128], F32, kind="ExternalInput")
    scal_d = nc.dram_tensor("scal", [128, 4], F32, kind="ExternalInput")
    h0_d = nc.dram_tensor("h0t", [128, 2 * BSR], F32, kind="ExternalInput")
    y_d = nc.dram_tensor("y", [2, 128, TBR], F32, kind="ExternalOutput")

    with TileContext(nc) as tc:
        with (
            tc.tile_pool(name="persist", bufs=1) as pp,
            tc.tile_pool(name="xstream", bufs=4) as xp,
            tc.tile_pool(name="rec_psum", bufs=1, space="PSUM") as rec_ps,
            tc.tile_pool(name="pre_psum", bufs=4, space="PSUM") as pre_ps,
        ):
            t_sb = [pp.tile([128, 256], F32, tag=f"t{i}", name=f"t{i}")
                    for i in range(ND)]
            b_sb = [pp.tile([128, 128], F32, tag=f"b{k}", name=f"b{k}")
                    for k in range(2)]
            scal_sb = pp.tile([128, 4], F32, tag="scal", name="scal_sb")
            h0_sb = pp.tile([128, 2 * BSR], F32, tag="h0t", name="h0_sb")
            xtt = [pp.tile([128, TBR], F32, tag=f"xtt{g}", name=f"xtt{g}")
                   for g in range(2)]
            ybuf = [pp.tile([128, TBR], F32, tag=f"ybuf{g}", name=f"ybuf{g}")
                    for g in range(2)]
            # psum [128, 16] tiles each occupy one 2KB bank; 4 + pre_ps 4 = 8
            ps_rec = [[rec_ps.tile([128, BSR], F32, tag=f"pr{k}_{i}",
                                   name=f"pr{k}_{i}") for i in range(2)]
                      for k in range(2)]

            mv = [scal_sb[:, k:k + 1] for k in range(2)]
            cv = [scal_sb[:, 2 + k:3 + k] for k in range(2)]

            # ---- small input DMAs -------------------------------------
            for i in range(ND):
                nc.sync.dma_start(out=t_sb[i][:],
                                  in_=t_d[128 * i:128 * (i + 1), :])
            for k in range(2):
                nc.sync.dma_start(out=b_sb[k][:],
                                  in_=b_d[128 * k:128 * (k + 1), :])
            nc.sync.dma_start(out=scal_sb[:], in_=scal_d[:, :])
            nc.sync.dma_start(out=h0_sb[:], in_=h0_d[:, :])

            # ---- phase 1: stream x in, project per unit chunk ---------
            for bj in range(NJR):
                xs = [xp.tile([128, 512], F32, tag=f"xs{i}",
                              name=f"xs_t{i}") for i in range(ND)]
                for i in range(ND):
                    nc.sync.dma_start(
                        out=xs[i][:],
                        in_=xtr_d[128 * i:128 * (i + 1),
                                  512 * bj:512 * (bj + 1)])
                for g in range(2):
                    ps = pre_ps.tile([128, 512], F32, tag="pre",
                                     name="pre_ps_t")
                    for i in range(ND):
                        nc.tensor.matmul(
                            ps[:],
                            t_sb[i][:, 128 * g:128 * (g + 1)],
                            xs[i][:],
                            start=(i == 0),
                            stop=(i == ND - 1),
                        )
                    nc.scalar.copy(
                        xtt[g][:, 512 * bj:512 * (bj + 1)], ps[:])

            # ---- recurrence: PE rot + DVE modrelu, 2 chains -----------
            for t in range(SEQ):
                cols = slice(BSR * t, BSR * (t + 1))
                for k in range(2):
                    rhs = (h0_sb[:, BSR * k:BSR * (k + 1)] if t == 0
                           else ybuf[k][:, BSR * (t - 1):BSR * t])
                    nc.tensor.matmul(ps_rec[k][t % 2][:], b_sb[k][:], rhs,
                                     start=True, stop=True)
                for k in range(2):
                    nc.vector._custom_dve(
                        MODRELU, out=ybuf[k][:, cols],
                        in0=ps_rec[k][t % 2][:], in1=xtt[k][:, cols],
                        s0=mv[k], s1=cv[k])
            for blk in range(NJR):
                lo, hi = 512 * blk, 512 * (blk + 1)
                for g in range(2):
                    nc.sync.dma_start(out=y_d[g, :, lo:hi],
                                      in_=ybuf[g][:, lo:hi])

    return nc


def _pack_inputs_pe(x, T, B, bias, h0):
    maps = []
    for c in range(NCORES):
        cb, cp = divmod(c, NBS)
        u0 = 256 * cp                                     # first unit
        Tp = np.ascontiguousarray(T[:, u0:u0 + 256])      # [DIN, 256]
        # the two [128,128] diagonal blocks of B for this unit range,
        # stacked: bmat[128k + i, j] = B[u0+128k+i, u0+128k+j]
        bm = np.concatenate(
            [B[u0 + 128 * k:u0 + 128 * (k + 1),
               u0 + 128 * k:u0 + 128 * (k + 1)] for k in range(2)], axis=0)
        bm = np.ascontiguousarray(bm)

        bs = bias[u0:u0 + 256].reshape(2, 128)
        scal = np.zeros((128, 4), dtype=np.float32)
        for k in range(2):
            scal[:, k] = np.where(bs[k] >= 0, np.float32(1e20),
                                  np.float32(-1.0))
            scal[:, 2 + k] = np.abs(bs[k])

        h0t = np.float32(1) * (np.concatenate(
            [np.repeat(h0[u0 + 128 * k:u0 + 128 * (k + 1)][:, None],
                       BSR, axis=1) for k in range(2)], axis=1))

        xs = x[cb * BSR:(cb + 1) * BSR]                   # [BSR, SEQ, DIN]
        xtr = np.ascontiguousarray(
            xs.transpose(2, 1, 0).reshape(DIN, TBR))      # [DIN, (t, b)]
        maps.append({"tmat": Tp, "bmat": bm, "scal": scal,
                     "h0t": np.ascontiguousarray(h0t), "xtr": xtr})
    return maps


def _unpack_pe(res):
    out = np.empty((SEQ, BATCH, UNITS), dtype=np.float32)
    for c in range(NCORES):
        cb, cp = divmod(c, NBS)
        y = res.results[c]["y"].reshape(2, 128, SEQ, BSR)
        # out[t, BSR*cb+b, 256cp+128k+u'] = y[k, u', t, b]
        tmp = y.transpose(2, 3, 0, 1).reshape(SEQ, BSR, 256)
        out[:, cb * BSR:(cb + 1) * BSR, 256 * cp:256 * (cp + 1)] = tmp
    return out


def _build_nc_rot():
    nc = bacc.Bacc()

    xtr_d = nc.dram_tensor("xtr", [DIN, TBR], F32, kind="ExternalInput")
    t_d = nc.dram_tensor("tmat", [DIN, 256], F32, kind="ExternalInput")
    scal_d = nc.dram_tensor("scal", [128, 8], F32, kind="ExternalInput")
    h0_d = nc.dram_tensor("h0t", [128, 2 * BSR], F32, kind="ExternalInput")
    y_d = nc.dram_tensor("y", [2, 128, TBR], F32, kind="ExternalOutput")

    with TileContext(nc) as tc:
        with (
            tc.tile_pool(name="persist", bufs=1) as pp,
            tc.tile_pool(name="xstream", bufs=4) as xp,
            tc.tile_pool(name="pre_psum", bufs=4, space="PSUM") as pre_ps,
        ):
            t_sb = [pp.tile([128, 256], F32, tag=f"t{i}", name=f"t{i}")
                    for i in range(ND)]
            scal_sb = pp.tile([128, 8], F32, tag="scal", name="scal_sb")
            h0_sb = pp.tile([128, 2 * BSR], F32, tag="h0t", name="h0_sb")
            xtt = [pp.tile([128, TBR], F32, tag=f"xtt{g}", name=f"xtt{g}")
                   for g in range(2)]
            ybuf = [pp.tile([128, TBR], F32, tag=f"ybuf{g}", name=f"ybuf{g}")
                    for g in range(2)]
            zt = [pp.tile([128, BSR], F32, tag=f"z{g}", name=f"z{g}")
                  for g in range(2)]

            # scal columns: 0..3 = A00, A10, A01, A11;
            # 4,5 = m_e, m_o; 6,7 = cb_e, cb_o
            A = [scal_sb[:, k:k + 1] for k in range(4)]
            mv = [scal_sb[:, 4 + g:5 + g] for g in range(2)]
            cv = [scal_sb[:, 6 + g:7 + g] for g in range(2)]

            # ---- small input DMAs -------------------------------------
            for i in range(ND):
                nc.sync.dma_start(out=t_sb[i][:],
                                  in_=t_d[128 * i:128 * (i + 1), :])
            nc.sync.dma_start(out=scal_sb[:], in_=scal_d[:, :])
            nc.sync.dma_start(out=h0_sb[:], in_=h0_d[:, :])

            # ---- phase 1 block emitter (interleaved with recurrence) --
            # xtt[g][q, col] = sum_d Tp[d, 128g+q] * x[d, col]
            def phase1_block(bj):
                xs = [xp.tile([128, 512], F32, tag=f"xs{i}",
                              name=f"xs_t{i}") for i in range(ND)]
                for i in range(ND):
                    nc.sync.dma_start(
                        out=xs[i][:],
                        in_=xtr_d[128 * i:128 * (i + 1),
                                  512 * bj:512 * (bj + 1)])
                for g in range(2):
                    ps = pre_ps.tile([128, 512], F32, tag="pre",
                                     name="pre_ps_t")
                    for i in range(ND):
                        nc.tensor.matmul(
                            ps[:],
                            t_sb[i][:, 128 * g:128 * (g + 1)],
                            xs[i][:],
                            start=(i == 0),
                            stop=(i == ND - 1),
                        )
                    nc.scalar.copy(
                        xtt[g][:, 512 * bj:512 * (bj + 1)], ps[:])

            for bj in range(2):
                phase1_block(bj)

            # ---- recurrence: 4 DVE ops per step, all SBUF -------------
            for t in range(SEQ):
                if t % 32 == 0 and t // 32 + 2 < NJR:
                    phase1_block(t // 32 + 2)
                cols = slice(BSR * t, BSR * (t + 1))
                if t == 0:
                    we = h0_sb[:, 0:BSR]
                    wo = h0_sb[:, BSR:2 * BSR]
                else:
                    we = ybuf[0][:, BSR * (t - 1):BSR * t]
                    wo = ybuf[1][:, BSR * (t - 1):BSR * t]
                nc.vector._custom_dve(
                    ROT, out=zt[0][:], in0=we, in1=wo, s0=A[0], s1=A[1])
                nc.vector._custom_dve(
                    ROT, out=zt[1][:], in0=we, in1=wo, s0=A[2], s1=A[3])
                for g in range(2):
                    nc.vector._custom_dve(
                        MODRELU, out=ybuf[g][:, cols],
                        in0=zt[g][:], in1=xtt[g][:, cols],
                        s0=mv[g], s1=cv[g])
            # drain the archive to HBM (after the loop: avoids per-tile
            # WAR dependencies between block DMAs and later state writes)
            for blk in range(NJR):
                lo, hi = 512 * blk, 512 * (blk + 1)
                for g in range(2):
                    nc.sync.dma_start(out=y_d[g, :, lo:hi],
                                      in_=ybuf[g][:, lo:hi])

    return nc


def _build_nc_rot2():
    """Like _build_nc_rot, but the recurrence uses 2 native
    scalar_tensor_tensor ops + 2 fused prescale-modrelu custom ops:
        t_e = we + (A10/A00)*wo            (native STT)
        t_o = wo + (A01/A11)*we            (native STT)
        we' = modrelu(A00*t_e + xtt_e, be) (MRS custom, signed bias)
        wo' = modrelu(A11*t_o + xtt_o, bo) (MRS custom)
    Natives are ~70ns cheaper than customs; requires |A00|,|A11| bounded
    away from 0 (guaranteed for rotation blocks with cos >= ~0.5)."""
    nc = bacc.Bacc()

    xtr_d = nc.dram_tensor("xtr", [DIN, TBR], F32, kind="ExternalInput")
    t_d = nc.dram_tensor("tmat", [DIN, 256], F32, kind="ExternalInput")
    scal_d = nc.dram_tensor("scal", [128, 6], F32, kind="ExternalInput")
    r0f_d = nc.dram_tensor("r0f", [128, BSR], F32, kind="ExternalInput")
    h0_d = nc.dram_tensor("h0t", [128, 2 * BSR], F32, kind="ExternalInput")
    y_d = nc.dram_tensor("y", [2, 128, TBR], F32, kind="ExternalOutput")

    with TileContext(nc) as tc:
        with (
            tc.tile_pool(name="persist", bufs=1) as pp,
            tc.tile_pool(name="xstream", bufs=4) as xp,
            tc.tile_pool(name="pre_psum", bufs=4, space="PSUM") as pre_ps,
        ):
            t_sb = [pp.tile([128, 256], F32, tag=f"t{i}", name=f"t{i}")
                    for i in range(ND)]
            scal_sb = pp.tile([128, 6], F32, tag="scal", name="scal_sb")
            h0_sb = pp.tile([128, 2 * BSR], F32, tag="h0t", name="h0_sb")
            xtt = [pp.tile([128, TBR], F32, tag=f"xtt{g}", name=f"xtt{g}")
                   for g in range(2)]
            ybuf = [pp.tile([128, TBR], F32, tag=f"ybuf{g}", name=f"ybuf{g}")
                    for g in range(2)]
            zt = [pp.tile([128, BSR], F32, tag=f"z{g}", name=f"z{g}")
                  for g in range(2)]
            r0f_sb = pp.tile([128, BSR], F32, tag="r0f", name="r0f_sb")
            zp_sb = pp.tile([128, BSR], F32, tag="zp", name="zp_sb")

            r0 = scal_sb[:, 0:1]
            r1 = scal_sb[:, 1:2]
            d0 = scal_sb[:, 2:3]
            d1 = scal_sb[:, 3:4]
            be = scal_sb[:, 4:5]
            bo = scal_sb[:, 5:6]

            for i in range(ND):
                nc.sync.dma_start(out=t_sb[i][:],
                                  in_=t_d[128 * i:128 * (i + 1), :])
            nc.sync.dma_start(out=scal_sb[:], in_=scal_d[:, :])
            nc.sync.dma_start(out=r0f_sb[:], in_=r0f_d[:, :])
            nc.sync.dma_start(out=h0_sb[:], in_=h0_d[:, :])

            def phase1_block(bj, nsub=1):
                # nsub > 1 splits the 512-col block into finer DMA/matmul/
                # copy chains so the first recurrence steps start sooner
                w = 512 // nsub
                for s in range(nsub):
                    lo = 512 * bj + w * s
                    xs = [xp.tile([128, w], F32, tag=f"xs{i}_{nsub}",
                                  name=f"xs_t{i}_{nsub}") for i in range(ND)]
                    for i in range(ND):
                        nc.sync.dma_start(
                            out=xs[i][:],
                            in_=xtr_d[128 * i:128 * (i + 1), lo:lo + w])
                    for g in range(2):
                        ps = pre_ps.tile([128, w], F32, tag=f"pre{nsub}",
                                         name="pre_ps_t")
                        for i in range(ND):
                            nc.tensor.matmul(
                                ps[:],
                                t_sb[i][:, 128 * g:128 * (g + 1)],
                                xs[i][:],
                                start=(i == 0),
                                stop=(i == ND - 1),
                            )
                        nc.scalar.copy(xtt[g][:, lo:lo + w], ps[:])

            for bj in range(2):
                phase1_block(bj, nsub=4)

            MUL = mybir.AluOpType.mult
            ADD = mybir.AluOpType.add
            for t in range(SEQ):
                if t % 32 == 0 and t // 32 + 2 < NJR:
                    phase1_block(t // 32 + 2)
                cols = slice(BSR * t, BSR * (t + 1))
                if t == 0:
                    we = h0_sb[:, 0:BSR]
                    wo = h0_sb[:, BSR:2 * BSR]
                else:
                    we = ybuf[0][:, BSR * (t - 1):BSR * t]
                    wo = ybuf[1][:, BSR * (t - 1):BSR * t]
                # t_e on the otherwise-idle GpSimd engine (plain
                # TensorTensor ops only - codegen rejects Pool STT):
                # it has the STT_o + MRS_o window before MRS_e needs it
                nc.gpsimd.tensor_tensor(out=zp_sb[:], in0=wo,
                                        in1=r0f_sb[:], op=MUL)
                nc.gpsimd.tensor_tensor(out=zt[0][:], in0=zp_sb[:],
                                        in1=we, op=ADD)
                nc.vector.scalar_tensor_tensor(
                    out=zt[1][:], in0=we, scalar=r1, in1=wo,
                    op0=MUL, op1=ADD)
                nc.vector._custom_dve(
                    MRS, out=ybuf[1][:, cols],
                    in0=zt[1][:], in1=xtt[1][:, cols], s0=d1, s1=bo)
                nc.vector._custom_dve(
                    MRS, out=ybuf[0][:, cols],
                    in0=zt[0][:], in1=xtt[0][:, cols], s0=d0, s1=be)
                if (t + 1) % 32 == 0:
                    blk = (t + 1) // 32 - 1
                    lo, hi = 512 * blk, 512 * (blk + 1)
                    for g in range(2):
                        nc.sync.dma_start(out=y_d[g, :, lo:hi],
                                          in_=ybuf[g][:, lo:hi])

    return nc


def _pack_inputs_rot2(x, T, B, bias, h0):
    p = np.arange(NPAIR)
    A00 = B[2 * p, 2 * p]
    A10 = B[2 * p + 1, 2 * p]
    A01 = B[2 * p, 2 * p + 1]
    A11 = B[2 * p + 1, 2 * p + 1]

    maps = []
    for c in range(NCORES):
        cb, cp = divmod(c, NBS)
        q = np.arange(128)
        pe = 128 * cp + q
        ue, uo = 2 * pe, 2 * pe + 1

        Tp = np.ascontiguousarray(
            np.concatenate([T[:, ue], T[:, uo]], axis=1))

        scal = np.zeros((128, 6), dtype=np.float32)
        scal[:, 0] = A10[pe] / A00[pe]
        scal[:, 1] = A01[pe] / A11[pe]
        scal[:, 2] = A00[pe]
        scal[:, 3] = A11[pe]
        scal[:, 4] = bias[ue]
        scal[:, 5] = bias[uo]

        h0t = np.concatenate(
            [np.repeat(h0[uu][:, None], BSR, axis=1) for uu in (ue, uo)],
            axis=1).astype(np.float32)
        r0f = np.repeat(scal[:, 0:1], BSR, axis=1).astype(np.float32)

        xs = x[cb * BSR:(cb + 1) * BSR]
        xtr = np.ascontiguousarray(
            xs.transpose(2, 1, 0).reshape(DIN, TBR))
        maps.append({"tmat": Tp, "scal": scal,
                     "r0f": np.ascontiguousarray(r0f),
                     "h0t": np.ascontiguousarray(h0t), "xtr": xtr})
    return maps


def _pack_inputs_rot(x, T, B, bias, h0):
    p = np.arange(NPAIR)
    A00 = B[2 * p, 2 * p]
    A10 = B[2 * p + 1, 2 * p]
    A01 = B[2 * p, 2 * p + 1]
    A11 = B[2 * p + 1, 2 * p + 1]

    maps = []
    for c in range(NCORES):
        cb, cp = divmod(c, NBS)
        q = np.arange(128)
        pe = 128 * cp + q                                 # pair indices
        ue, uo = 2 * pe, 2 * pe + 1                       # unit indices

        Tp = np.ascontiguousarray(
            np.concatenate([T[:, ue], T[:, uo]], axis=1))  # [DIN, 256]

        scal = np.zeros((128, 8), dtype=np.float32)
        scal[:, 0] = A00[pe]
        scal[:, 1] = A10[pe]
        scal[:, 2] = A01[pe]
        scal[:, 3] = A11[pe]
        for g, uu in enumerate((ue, uo)):
            scal[:, 4 + g] = np.where(bias[uu] >= 0, np.float32(1e20),
                                      np.float32(-1.0))
            scal[:, 6 + g] = np.abs(bias[uu])

        h0t = np.concatenate(
            [np.repeat(h0[uu][:, None], BSR, axis=1) for uu in (ue, uo)],
            axis=1).astype(np.float32)                    # [128, 32]

        xs = x[cb * BSR:(cb + 1) * BSR]                   # [BSR, SEQ, DIN]
        xtr = np.ascontiguousarray(
            xs.transpose(2, 1, 0).reshape(DIN, TBR))      # [DIN, (t, b)]
        maps.append({"tmat": Tp, "scal": scal,
                     "h0t": np.ascontiguousarray(h0t), "xtr": xtr})
    return maps


def _unpack_rot(res):
    out = np.empty((SEQ, BATCH, UNITS), dtype=np.float32)
    q = np.arange(128)
    for c in range(NCORES):
        cb, cp = divmod(c, NBS)
        pe = 128 * cp + q
        y = res.results[c]["y"].reshape(2, 128, SEQ, BSR)
        # out[t, 16cb+b, unit(g, q)] = y[g, q, t, b]
        tmp = y.transpose(2, 3, 0, 1)                     # [SEQ, BSR, 2, 128]
        bsl = slice(cb * BSR, (cb + 1) * BSR)
        out[:, bsl, 2 * pe] = tmp[:, :, 0, :]
        out[:, bsl, 2 * pe + 1] = tmp[:, :, 1, :]
    return out


def _is_block_diag(B):
    Babs = np.abs(B)
    mask = np.zeros((UNITS, UNITS), dtype=bool)
    p = np.arange(NPAIR)
    for (i, j) in ((0, 0), (0, 1), (1, 0), (1, 1)):
        mask[2 * p + i, 2 * p + j] = True
    off = Babs[~mask]
    return off.max() <= 1e-5 * max(Babs.max(), 1e-30)


# ---------------------------------------------------------------------------
# Fallback: dense f32r split-precision matmul recurrence (previous kernel)
# ---------------------------------------------------------------------------

def _build_nc_dense(repeat=1):
    nc = bacc.Bacc()

    xtr_d = nc.dram_tensor("xtr", [DIN, TB], F32, kind="ExternalInput")
    t_d = nc.dram_tensor("tmat", [DIN, UNITS], F32, kind="ExternalInput")
    b2_d = nc.dram_tensor("b2", [2 * UNITS, UNITS], F32, kind="ExternalInput")
    h02_d = nc.dram_tensor("h02", [UNITS, 2 * BS], F32, kind="ExternalInput")
    mv_d = nc.dram_tensor("mv", [UNITS, 1], F32, kind="ExternalInput")
    cv_d = nc.dram_tensor("cv", [UNITS, 1], F32, kind="ExternalInput")
    y_d = nc.dram_tensor("y", [NK, 128, TB], F32, kind="ExternalOutput")

    with TileContext(nc) as tc:
        with (
            tc.tile_pool(name="persist", bufs=1) as pp,
            tc.tile_pool(name="pre_psum", bufs=4, space="PSUM") as pre_ps,
            tc.tile_pool(name="rec_psum", bufs=1, space="PSUM") as rec_ps,
        ):
            xtr_sb = [pp.tile([128, TB], F32, tag=f"xtr{i}", name=f"xtr{i}")
                      for i in range(ND)]
            t_sb = [pp.tile([128, UNITS], F32, tag=f"t{i}", name=f"t{i}")
                    for i in range(ND)]
            b2_sb = [pp.tile([128, UNITS], F32R, tag=f"b2_{j}", name=f"b2_{j}")
                     for j in range(2 * NK)]
            h02_sb = pp.tile([128, 2 * BS * NK], F32R, tag="h02", name="h02")
            mv_sb = pp.tile([128, NK], F32, tag="mv", name="mv_sb")
            cv_sb = pp.tile([128, NK], F32, tag="cv", name="cv_sb")
            xtt_all = pp.tile([128, NK * TB], F32, tag="xtt", name="xtt_all")
            ybuf_all = pp.tile([128, NK * TB], F32, tag="ybuf", name="ybuf_all")
            sb2_all = pp.tile([128, 4 * 32], F32R, tag="sb2", name="sb2_all")
            zt_pp = [pp.tile([128, 128], F32, tag=f"zt{i}", name=f"zt{i}")
                     for i in range(2)]
            zs_pp = [pp.tile([128, 32], F32, tag=f"zs{i}", name=f"zs{i}")
                     for i in range(2)]
            ps_pp = [[rec_ps.tile([32, UNITS // 2], F32, tag=f"ps{i}_{h}",
                                  name=f"ps{i}_{h}") for h in range(2)]
                     for i in range(2)]

            xtr_v = [xtr_sb[i][:] for i in range(ND)]
            t_v = [t_sb[i][:] for i in range(ND)]
            bhi_v = [b2_sb[k][:] for k in range(NK)]
            blo_v = [b2_sb[NK + k][:] for k in range(NK)]
            h0_v = [h02_sb[:, 2 * k * BS:2 * (k + 1) * BS] for k in range(NK)]
            mv_v = [mv_sb[:, k:k + 1] for k in range(NK)]
            cv_v = [cv_sb[:, k:k + 1] for k in range(NK)]

            for i in range(ND):
                nc.sync.dma_start(out=xtr_sb[i][:], in_=xtr_d[128 * i:128 * (i + 1), :])
                nc.sync.dma_start(out=t_sb[i][:], in_=t_d[128 * i:128 * (i + 1), :])
            for j in range(2 * NK):
                nc.sync.dma_start(out=b2_sb[j][:],
                                  in_=b2_d[128 * j:128 * (j + 1), :].bitcast(F32R))
            for k in range(NK):
                nc.sync.dma_start(
                    out=h02_sb[:, 2 * k * BS:2 * (k + 1) * BS],
                    in_=h02_d[128 * k:128 * (k + 1), :].bitcast(F32R))
                nc.sync.dma_start(out=mv_sb[:, k:k + 1], in_=mv_d[128 * k:128 * (k + 1), :])
                nc.sync.dma_start(out=cv_sb[:, k:k + 1], in_=cv_d[128 * k:128 * (k + 1), :])

            for i in range(2):
                for h in range(2):
                    nc.vector.memset(ps_pp[i][h][:], 0.0)
            warm_sb = pp.tile([128, 2], F32, tag="warm_sb", name="warm_sb")
            nc.vector.tensor_copy(warm_sb[:, 0:1], mv_v[0])
            nc.vector.tensor_copy(warm_sb[:, 1:2], cv_v[0])

            NJ = TB // 512
            for m in range(NK):
                for j in range(NJ):
                    ps = pre_ps.tile([128, 512], F32, tag="pre", name="pre_ps_t")
                    for i in range(ND):
                        nc.tensor.matmul(
                            ps[:],
                            t_v[i][:, 128 * m:128 * (m + 1)],
                            xtr_v[i][:, 512 * j:512 * (j + 1)],
                            start=(i == 0),
                            stop=(i == ND - 1),
                        )
                    nc.vector.tensor_copy(
                        xtt_all[:, TB * m + 512 * j:TB * m + 512 * (j + 1)], ps[:])

            import contextlib
            loop_cm = (tc.For_i(0, repeat, 1) if repeat > 1
                       else contextlib.nullcontext())
            with loop_cm:
              for t in range(SEQ):
                  zt = zt_pp[t % 2]
                  zs = zs_pp[t % 2]
                  pprev = (t - 1) % 2
                  p = t % 2
                  zt_v = zt[:].rearrange("p (k c) -> p k c", k=NK)
                  zs_v = zs[:].rearrange("p (k c) -> p k c", k=NK)
                  yb_v = ybuf_all[:].rearrange("p (k c) -> p k c", k=NK)
                  sb_v = sb2_all[:].rearrange("p (k c) -> p k c", k=NK)

                  def split_state(ks):
                      yv = yb_v[:, ks, BS * t:BS * (t + 1)]
                      hv = sb_v[:, ks, 16 * p:16 * p + BS]
                      lv = sb_v[:, ks, 16 * p + BS:16 * p + 2 * BS]
                      nc.vector.tensor_copy(hv, yv)
                      nc.vector.tensor_sub(lv, yv, hv.bitcast(F32))

                  for h in range(2):
                      ps = ps_pp[t % 2][h]
                      cols = slice(256 * h, 256 * (h + 1))
                      for k in range(NK):
                          lhsT = (h0_v[k] if t == 0
                                  else sb2_all[:, 32 * k + 16 * pprev:
                                               32 * k + 16 * pprev + 16])
                          for i, bp_v in enumerate((bhi_v, blo_v)):
                              nc.tensor.matmul(
                                  ps[0:2 * BS, :],
                                  lhsT,
                                  bp_v[k][:, cols],
                                  start=(k == 0 and i == 0),
                                  stop=(k == NK - 1 and i == 1),
                              )
                      ps_v = ps[:].rearrange("p (k c) -> p k c", k=2)
                      for q in range(4):
                          in_ap = ps_v[:, :, 32 * q:32 * (q + 1)]
                          out_ap = zt[32 * q:32 * (q + 1), :].rearrange(
                              "p (k c) -> p k c", k=NK)[:, 2 * h:2 * h + 2, :]
                          nc.vector.transpose(out_ap, in_ap)
                      hs = slice(2 * h, 2 * h + 2)
                      nc.vector.tensor_add(
                          zs_v[:, hs, :], zt_v[:, hs, 0:BS],
                          zt_v[:, hs, BS:2 * BS])
                      for k in (2 * h, 2 * h + 1):
                          nc.vector._custom_dve(
                              MODRELU,
                              out=ybuf_all[:, TB * k + BS * t:TB * k + BS * (t + 1)],
                              in0=zs[:, BS * k:BS * (k + 1)],
                              in1=xtt_all[:, TB * k + BS * t:TB * k + BS * (t + 1)],
                              s0=mv_v[k],
                              s1=cv_v[k],
                          )
                      split_state(hs)
                  if (t + 1) % 128 == 0:
                      blk = (t + 1) // 128 - 1
                      lo, hi = 1024 * blk, 1024 * (blk + 1)
                      for k in range(NK):
                          nc.sync.dma_start(
                              out=y_d[k, :, lo:hi],
                              in_=ybuf_all[:, TB * k + lo:TB * k + hi],
                          )

    return nc


def _round_f32r(a):
    u = np.ascontiguousarray(a, dtype=np.float32).view(np.uint32).copy()
    u += np.uint32(0x7FF) + ((u >> np.uint32(12)) & np.uint32(1))
    u &= np.uint32(0xFFFFF000)
    return u.view(np.float32)


def _pack_inputs_dense(x, T, B, bias, h0):
    Bhi = _round_f32r(B)
    Blo = _round_f32r(B - Bhi)
    b2 = np.concatenate([Bhi, Blo], axis=0)
    mv = np.where(bias >= 0, np.float32(1e20), np.float32(-1.0)).astype(np.float32)
    cv = np.abs(bias).astype(np.float32)
    h0b = np.repeat(h0[:, None], BS, axis=1).astype(np.float32)
    h0hi = _round_f32r(h0b)
    h0lo = _round_f32r(h0b - h0hi)
    h02 = np.concatenate([h0hi, h0lo], axis=1)

    base = {
        "tmat": T,
        "b2": b2,
        "h02": np.ascontiguousarray(h02),
        "mv": mv.reshape(UNITS, 1),
        "cv": cv.reshape(UNITS, 1),
    }
    maps = []
    for c in range(NCORES):
        xs = x[c * BS:(c + 1) * BS]
        xtr = np.ascontiguousarray(
            xs.transpose(2, 1, 0).reshape(DIN, TB))
        m = dict(base)
        m["xtr"] = xtr
        maps.append(m)
    return maps


def _unpack_dense(res):
    out = np.empty((SEQ, BATCH, UNITS), dtype=np.float32)
    for c in range(NCORES):
        y = res.results[c]["y"].reshape(NK, 128, SEQ, BS)
        out[:, c * BS:(c + 1) * BS, :] = (
            y.transpose(2, 3, 0, 1).reshape(SEQ, BS, UNITS))
    return out


# ---------------------------------------------------------------------------

_BUILDERS = {
    "pe": _build_nc_rot_pe,
    "dve": _build_nc_rot,
    "dve2": _build_nc_rot2,
    "dense": _build_nc_dense,
}


def _get_nc(kind):
    if kind not in _NC_CACHE:
        nc = _BUILDERS[kind]()
        nc.finalize()
        _NC_CACHE[kind] = nc
    return _NC_CACHE[kind]


def kernel(x, T, B, bias, h0):
    """Full-input, full-output entry point."""
    global LAST_RESULTS
    x = np.ascontiguousarray(np.asarray(x, dtype=np.float32))
    T = np.ascontiguousarray(np.asarray(T, dtype=np.float32))
    B = np.ascontiguousarray(np.asarray(B, dtype=np.float32))
    bias = np.asarray(bias, dtype=np.float32)
    h0 = np.asarray(h0, dtype=np.float32)

    if _is_block_diag(B):
        p = np.arange(NPAIR)
        dmin = min(np.abs(B[2 * p, 2 * p]).min(),
                   np.abs(B[2 * p + 1, 2 * p + 1]).min())
        kind = os.environ.get("KERNEL_IMPL",
                              "dve2" if dmin > 0.05 else "dve")
    else:
        kind = "dense"
    if kind == "pe":
        in_maps = _pack_inputs_pe(x, T, B, bias, h0)
    elif kind == "dve":
        in_maps = _pack_inputs_rot(x, T, B, bias, h0)
    elif kind == "dve2":
        in_maps = _pack_inputs_rot2(x, T, B, bias, h0)
    else:
        in_maps = _pack_inputs_dense(x, T, B, bias, h0)

    nc = _get_nc(kind)
    trace = bool(int(os.environ.get("KERNEL_TRACE", "0")))
    res = bass_utils.run_bass_kernel_spmd(
        nc, in_maps, list(range(NCORES)), trace=trace)
    LAST_RESULTS = res

    if kind == "pe":
        return _unpack_pe(res)
    if kind in ("dve", "dve2"):
        return _unpack_rot(res)
    return _unpack_dense(res)


if __name__ == "__main__":
    rng = np.random.default_rng(0)
    x = rng.standard_normal((BATCH, SEQ, DIN), dtype=np.float32)
    T = rng.standard_normal((DIN, UNITS), dtype=np.float32) / DIN
    # block-diagonal rotation B
    th = rng.uniform(0, np.pi / 2, NPAIR).astype(np.float32)
    B = np.zeros((UNITS, UNITS), dtype=np.float32)
    p = np.arange(NPAIR)
    B[2 * p, 2 * p] = np.cos(th)
    B[2 * p, 2 * p + 1] = np.sin(th)
    B[2 * p + 1, 2 * p] = -np.sin(th)
    B[2 * p + 1, 2 * p + 1] = np.cos(th)
    bias = rng.uniform(-0.01, 0.01, UNITS).astype(np.float32)
    h0 = np.zeros(UNITS, dtype=np.float32)
    out = kernel(x=x, T=T, B=B, bias=bias, h0=h0)
    print("out", out.shape, out.dtype, float(np.abs(out).mean()))
